# revision 6
# baseline (speedup 1.0000x reference)
"""Trainium2 Bass kernel for nn_Node_Transformation.

Reference semantics, for row n:
    out[n] = x[n] @ W.T + b            if node_type[n] == item_id
             emb_weight[node_type[n]]  otherwise

Only ~1/8 of rows take the linear path, so the kernel is split:

  Dense part (all rows): out_dense[n] = emb_weight[node_type[n]], computed as a
  one-hot matmul: outT[h, r] = sum_t table[t, h] * onehot[t, r], with the tiny
  table as the stationary operand (loaded once) and the host-built one-hot
  indicator streaming as rhs. Output is produced hid-major ("outT") so the
  per-group varying operand is the streaming one (no LDWEIGHTS churn).

  Sparse part (selected rows only): row indices where node_type == item_id are
  computed on host (metadata only); the kernel gathers just those x rows via
  indirect DMA (1/8 of x traffic), transposes them on the PE, and computes
  lin = x_sel @ W.T + b into a compact second output. The host scatters those
  rows over the dense result while unsharding.

Everything on-device is bf16 (psum accumulation in f32); the correctness gate
is a scale-relative 2e-2 absmax, bf16 error is ~4e-3.

Sharding: data-parallel over N across 8 NeuronCores; weights/table replicated.
"""

import os
import numpy as np
import ml_dtypes

import concourse.bass as bass
import concourse.bacc as bacc
import concourse.mybir as mybir
from concourse.tile import TileContext
from concourse.bass import IndirectOffsetOnAxis
from concourse.bass_utils import run_bass_kernel_spmd
from concourse.masks import make_identity

# ---- problem constants (hardcoded per contest contract) ----
N = 500000
IN_CH = 256
HID = 128
NUM_T = 8
NCORES = 8
NSH = N // NCORES          # 62500 rows per core
DGRP = 512                 # dense rows per matmul group (one f32 PSUM bank)
NG = (NSH + DGRP - 1) // DGRP          # 123 dense groups
PADR = NG * DGRP                       # 62976 padded rows per core
OHC = 8192                 # one-hot columns loaded per DMA (16 dense groups)
SLABG = 16                 # dense groups per output slab (8192 cols per DMA)
SGRP = 512                 # sel rows per matmul group (4 tiles, f32 psum bank)

BF16 = ml_dtypes.bfloat16

_CACHE = {}


def _ensure_axon_profile_hook():
    """bass_utils' trace path imports antenv.axon_hooks, which this image
    lacks. Register an equivalent module backed by the axon PJRT .so so
    trace=True (or BASS_TRACE=1) works instead of crashing."""
    try:
        import antenv.axon_hooks  # noqa: F401
        return
    except ImportError:
        pass
    import sys
    import types

    hook = None
    try:
        from trn_agent_boot.trn_boot import _ntff_profile_via_ctypes

        hook = _ntff_profile_via_ctypes("/opt/axon/libaxon_pjrt.so")
    except Exception:
        hook = None
    mod = types.ModuleType("antenv.axon_hooks")
    mod.get_axon_ntff_profile_hook = lambda: hook
    mod.set_axon_ntff_profile_hook = lambda h: None
    sys.modules["antenv.axon_hooks"] = mod
    try:
        import antenv

        antenv.axon_hooks = mod
    except ImportError:
        pass


def _build(tsel: int) -> bass.Bass:
    nc = bacc.Bacc("TRN2")
    f32 = mybir.dt.float32
    bf16 = mybir.dt.bfloat16
    i32 = mybir.dt.int32

    x_d = nc.dram_tensor("x", [NSH, IN_CH], bf16, kind="ExternalInput")
    oh_d = nc.dram_tensor("oh", [NUM_T, PADR], bf16, kind="ExternalInput")
    idx_d = nc.dram_tensor("idx", [128, tsel], i32, kind="ExternalInput")
    t2_d = nc.dram_tensor("t2", [NUM_T, HID], bf16, kind="ExternalInput")
    wt_d = nc.dram_tensor("wt", [IN_CH, HID], bf16, kind="ExternalInput")
    bb_d = nc.dram_tensor("bb", [HID, 1], f32, kind="ExternalInput")
    outT_d = nc.dram_tensor("outT", [HID, PADR], bf16, kind="ExternalOutput")
    o2T_d = nc.dram_tensor("o2T", [HID, tsel * 128], bf16, kind="ExternalOutput")

    sgroups = tsel // 4                 # sel tiles are processed 4 per group
    # spread the sel groups evenly through the dense loop
    sel_at = set(int(round((i + 0.5) * NG / sgroups)) for i in range(sgroups))
    assert len(sel_at) == sgroups

    with TileContext(nc) as tc:
        with (
            tc.tile_pool(name="singles", bufs=1) as singles,
            tc.tile_pool(name="ohp", bufs=2) as ohpool,
            tc.tile_pool(name="osl", bufs=3) as opool,
            tc.tile_pool(name="xsp", bufs=3) as xpool,
            tc.tile_pool(name="xtp", bufs=2) as xtpool,
            tc.tile_pool(name="o2p", bufs=2) as o2pool,
            tc.tile_pool(name="psd", bufs=3, space="PSUM") as psd,
            tc.tile_pool(name="pst", bufs=3, space="PSUM") as pst,
            tc.tile_pool(name="psl", bufs=2, space="PSUM") as psl,
        ):
            ident = singles.tile([128, 128], bf16)
            make_identity(nc, ident)

            t2_s = singles.tile([NUM_T, HID], bf16)
            nc.sync.dma_start(out=t2_s[:], in_=t2_d[:])
            wt_s = singles.tile([128, 2, HID], bf16)
            nc.sync.dma_start(out=wt_s[:], in_=wt_d[:].rearrange("(k c) h -> c k h", k=2))
            bb_s = singles.tile([HID, 1], f32)
            nc.sync.dma_start(out=bb_s[:], in_=bb_d[:])
            idx_s = singles.tile([128, tsel], i32)
            nc.sync.dma_start(out=idx_s[:], in_=idx_d[:])

            oh_tile = None
            oh_base = 0
            oslab = None
            slab_g0 = 0
            sel_emitted = 0

            def emit_sel_group(sg):
                xsT = xtpool.tile([128, 2, SGRP], bf16, tag="xsT")
                for j in range(4):
                    t = sg * 4 + j
                    xs = xpool.tile([128, IN_CH], bf16, tag="xs")
                    nc.gpsimd.indirect_dma_start(
                        out=xs[:], out_offset=None, in_=x_d[:],
                        in_offset=IndirectOffsetOnAxis(ap=idx_s[:, t : t + 1], axis=0),
                    )
                    pt = pst.tile([128, 2, 128], bf16, tag="pt")
                    nc.tensor.transpose(pt[:, 0, :], xs[:, 0:128], ident[:])
                    nc.tensor.transpose(pt[:, 1, :], xs[:, 128:256], ident[:])
                    nc.vector.tensor_copy(xsT[:, :, j * 128 : (j + 1) * 128], pt[:])
                lp = psl.tile([HID, SGRP], f32, tag="lp")
                nc.tensor.matmul(out=lp[:], lhsT=wt_s[:, 0, :], rhs=xsT[:, 0, :],
                                 start=True, stop=False)
                nc.tensor.matmul(out=lp[:], lhsT=wt_s[:, 1, :], rhs=xsT[:, 1, :],
                                 start=False, stop=True)
                o2 = o2pool.tile([HID, SGRP], bf16, tag="o2")
                nc.scalar.activation(out=o2[:], in_=lp[:],
                                     func=mybir.ActivationFunctionType.Identity,
                                     bias=bb_s[:, 0:1], scale=1.0)
                nc.scalar.dma_start(out=o2T_d[:, sg * SGRP : (sg + 1) * SGRP], in_=o2[:])

            for g in range(NG):
                c0 = g * DGRP
                if g % (OHC // DGRP) == 0:
                    oh_tile = ohpool.tile([NUM_T, OHC], bf16, tag="oh")
                    lo = g * DGRP
                    hi = min(lo + OHC, PADR)
                    nc.sync.dma_start(out=oh_tile[:, 0 : hi - lo], in_=oh_d[:, lo:hi])
                    oh_base = lo
                if g % SLABG == 0:
                    oslab = opool.tile([HID, SLABG * DGRP], bf16, tag="oslab")
                    slab_g0 = g

                pd = psd.tile([HID, DGRP], f32, tag="pd")
                nc.tensor.matmul(
                    out=pd[:], lhsT=t2_s[:],
                    rhs=oh_tile[:, c0 - oh_base : c0 - oh_base + DGRP],
                    start=True, stop=True,
                )
                so = (g - slab_g0) * DGRP
                if g % 2 == 0:
                    nc.vector.tensor_copy(oslab[:, so : so + DGRP], pd[:])
                else:
                    nc.scalar.activation(out=oslab[:, so : so + DGRP], in_=pd[:],
                                         func=mybir.ActivationFunctionType.Copy)

                if g == slab_g0 + SLABG - 1 or g == NG - 1:
                    lo = slab_g0 * DGRP
                    hi = (g + 1) * DGRP
                    nc.scalar.dma_start(out=outT_d[:, lo:hi], in_=oslab[:, 0 : hi - lo])

                if g in sel_at and sel_emitted < sgroups:
                    emit_sel_group(sel_emitted)
                    sel_emitted += 1

            while sel_emitted < sgroups:
                emit_sel_group(sel_emitted)
                sel_emitted += 1

    nc.compile()
    return nc


def _prepare(inputs):
    x = np.asarray(inputs["x"])
    nt = np.asarray(inputs["node_type"]).astype(np.int64)
    item = int(np.asarray(inputs["item_id"]))
    emb = np.asarray(inputs["emb_weight"], dtype=np.float32)
    W = np.asarray(inputs["W"], dtype=np.float32)
    b = np.asarray(inputs["b"], dtype=np.float32)

    t2 = emb.astype(BF16)
    wt = np.ascontiguousarray(W.T).astype(BF16)
    bb = b.astype(np.float32).reshape(HID, 1)

    sels = []
    max_nsel = 0
    for c in range(NCORES):
        sel = np.flatnonzero(nt[c * NSH : (c + 1) * NSH] == item).astype(np.int32)
        sels.append(sel)
        max_nsel = max(max_nsel, len(sel))
    tsel = max(64, ((-(-max_nsel // 128) + 3) // 4 + 1) * 4)

    in_maps = []
    for c in range(NCORES):
        nt_sh = nt[c * NSH : (c + 1) * NSH]
        xb = np.ascontiguousarray(x[c * NSH : (c + 1) * NSH]).astype(BF16)

        oh = np.zeros((NUM_T, PADR), dtype=BF16)
        for t in range(NUM_T):
            oh[t, :NSH] = (nt_sh == t)

        idxp = np.zeros(tsel * 128, dtype=np.int32)
        idxp[: len(sels[c])] = sels[c]
        idx_pm = np.ascontiguousarray(idxp.reshape(tsel, 128).T)

        in_maps.append({"x": xb, "oh": oh, "idx": idx_pm,
                        "t2": t2, "wt": wt, "bb": bb})
    return tsel, sels, in_maps


def _run(inputs, trace=False):
    _ensure_axon_profile_hook()
    tsel, sels, in_maps = _prepare(inputs)
    if tsel not in _CACHE:
        _CACHE[tsel] = _build(tsel)
    nc = _CACHE[tsel]
    res = run_bass_kernel_spmd(nc, in_maps, core_ids=list(range(NCORES)), trace=trace)
    out = np.empty((N, HID), np.float32)
    for c in range(NCORES):
        outT = res.results[c]["outT"]          # [HID, PADR] bf16
        osh = out[c * NSH : (c + 1) * NSH]
        osh[:] = outT[:, :NSH].astype(np.float32).T
        sel = sels[c]
        if len(sel):
            o2T = res.results[c]["o2T"]        # [HID, tsel*128] bf16
            osh[sel] = o2T[:, : len(sel)].astype(np.float32).T
    return out, res


def kernel(**inputs) -> np.ndarray:
    out, _ = _run(inputs, trace=bool(os.environ.get("KERNEL_TRACE")))
    return out


# revision 8
# speedup vs baseline: 1.2060x; 1.2060x over previous
"""Trainium2 Bass kernel for nn_Node_Transformation.

Reference semantics, for row n:
    out[n] = x[n] @ W.T + b            if node_type[n] == item_id
             emb_weight[node_type[n]]  otherwise

Only ~1/8 of rows take the linear path, so the kernel is split:

  Dense part (all rows): out_dense[n] = emb_weight[node_type[n]], computed as a
  one-hot matmul: outT[h, r] = sum_t table[t, h] * onehot[t, r], with the tiny
  table as the stationary operand (loaded once) and the host-built one-hot
  indicator streaming as rhs. Output is produced hid-major ("outT") so the
  per-group varying operand is the streaming one (no LDWEIGHTS churn).

  Sparse part (selected rows only): row indices where node_type == item_id are
  computed on host (metadata only); the kernel gathers just those x rows via
  indirect DMA (1/8 of x traffic), transposes them on the PE, and computes
  lin = x_sel @ W.T + b into a compact second output. The host scatters those
  rows over the dense result while unsharding.

Everything on-device is bf16 (psum accumulation in f32); the correctness gate
is a scale-relative 2e-2 absmax, bf16 error is ~4e-3.

Sharding: data-parallel over N across 8 NeuronCores; weights/table replicated.
"""

import os
import numpy as np
import ml_dtypes

import concourse.bass as bass
import concourse.bacc as bacc
import concourse.mybir as mybir
from concourse.tile import TileContext
from concourse.bass import IndirectOffsetOnAxis
from concourse.bass_utils import run_bass_kernel_spmd
from concourse.masks import make_identity

# ---- problem constants (hardcoded per contest contract) ----
N = 500000
IN_CH = 256
HID = 128
NUM_T = 8
NCORES = 8
NSH = N // NCORES          # 62500 rows per core
DGRP = 512                 # dense rows per matmul group (one f32 PSUM bank)
NG = (NSH + DGRP - 1) // DGRP          # 123 dense groups
PADR = NG * DGRP                       # 62976 padded rows per core
OHC = 8192                 # one-hot columns loaded per DMA (16 dense groups)
SLABG = 16                 # dense groups per output slab (8192 cols per DMA)
SGRP = 512                 # sel rows per matmul group (4 tiles, f32 psum bank)

BF16 = ml_dtypes.bfloat16

_CACHE = {}


def _ensure_axon_profile_hook():
    """bass_utils' trace path imports antenv.axon_hooks, which this image
    lacks. Register an equivalent module backed by the axon PJRT .so so
    trace=True (or BASS_TRACE=1) works instead of crashing."""
    try:
        import antenv.axon_hooks  # noqa: F401
        return
    except ImportError:
        pass
    import sys
    import types

    hook = None
    try:
        from trn_agent_boot.trn_boot import _ntff_profile_via_ctypes

        hook = _ntff_profile_via_ctypes("/opt/axon/libaxon_pjrt.so")
    except Exception:
        hook = None
    mod = types.ModuleType("antenv.axon_hooks")
    mod.get_axon_ntff_profile_hook = lambda: hook
    mod.set_axon_ntff_profile_hook = lambda h: None
    sys.modules["antenv.axon_hooks"] = mod
    try:
        import antenv

        antenv.axon_hooks = mod
    except ImportError:
        pass


def _build(tsel: int) -> bass.Bass:
    nc = bacc.Bacc("TRN2")
    f32 = mybir.dt.float32
    bf16 = mybir.dt.bfloat16
    i32 = mybir.dt.int32

    x_d = nc.dram_tensor("x", [NSH, IN_CH], bf16, kind="ExternalInput")
    oh_d = nc.dram_tensor("oh", [NUM_T, PADR], bf16, kind="ExternalInput")
    idx_d = nc.dram_tensor("idx", [128, tsel], i32, kind="ExternalInput")
    t2_d = nc.dram_tensor("t2", [NUM_T, HID], bf16, kind="ExternalInput")
    wt_d = nc.dram_tensor("wt", [IN_CH, HID], bf16, kind="ExternalInput")
    bb_d = nc.dram_tensor("bb", [HID, 1], f32, kind="ExternalInput")
    outT_d = nc.dram_tensor("outT", [HID, PADR], bf16, kind="ExternalOutput")
    o2T_d = nc.dram_tensor("o2T", [HID, tsel * 128], bf16, kind="ExternalOutput")

    sgroups = tsel // 4                 # sel tiles are processed 4 per group
    # spread the sel groups evenly through the dense loop
    sel_at = set(int(round((i + 0.5) * NG / sgroups)) for i in range(sgroups))
    assert len(sel_at) == sgroups

    with TileContext(nc) as tc:
        with (
            tc.tile_pool(name="singles", bufs=1) as singles,
            tc.tile_pool(name="ohp", bufs=3) as ohpool,
            tc.tile_pool(name="osl", bufs=3) as opool,
            tc.tile_pool(name="xsp", bufs=12) as xpool,
            tc.tile_pool(name="xtp", bufs=2) as xtpool,
            tc.tile_pool(name="o2p", bufs=2) as o2pool,
            tc.tile_pool(name="psd", bufs=3, space="PSUM") as psd,
            tc.tile_pool(name="pst", bufs=3, space="PSUM") as pst,
            tc.tile_pool(name="psl", bufs=2, space="PSUM") as psl,
        ):
            ident = singles.tile([128, 128], bf16)
            make_identity(nc, ident)

            t2_s = singles.tile([NUM_T, HID], bf16)
            nc.sync.dma_start(out=t2_s[:], in_=t2_d[:])
            wt_s = singles.tile([128, 2, HID], bf16)
            nc.sync.dma_start(out=wt_s[:], in_=wt_d[:].rearrange("(k c) h -> c k h", k=2))
            bb_s = singles.tile([HID, 1], f32)
            nc.sync.dma_start(out=bb_s[:], in_=bb_d[:])
            idx_s = singles.tile([128, tsel], i32)
            nc.sync.dma_start(out=idx_s[:], in_=idx_d[:])

            oh_tiles = {}
            oslab = None
            slab_g0 = 0
            gathered = {}          # sg -> list of 4 xs tiles (in-flight gathers)

            def emit_gathers(sg):
                tiles = []
                for j in range(4):
                    t = sg * 4 + j
                    xs = xpool.tile([128, IN_CH], bf16, tag="xs")
                    nc.gpsimd.indirect_dma_start(
                        out=xs[:], out_offset=None, in_=x_d[:],
                        in_offset=IndirectOffsetOnAxis(ap=idx_s[:, t : t + 1], axis=0),
                    )
                    tiles.append(xs)
                gathered[sg] = tiles

            def emit_sel_compute(sg):
                xsT = xtpool.tile([128, 2, SGRP], bf16, tag="xsT")
                for j, xs in enumerate(gathered.pop(sg)):
                    pt = pst.tile([128, 2, 128], bf16, tag="pt")
                    nc.tensor.transpose(pt[:, 0, :], xs[:, 0:128], ident[:])
                    nc.tensor.transpose(pt[:, 1, :], xs[:, 128:256], ident[:])
                    nc.vector.tensor_copy(xsT[:, :, j * 128 : (j + 1) * 128], pt[:])
                lp = psl.tile([HID, SGRP], f32, tag="lp")
                nc.tensor.matmul(out=lp[:], lhsT=wt_s[:, 0, :], rhs=xsT[:, 0, :],
                                 start=True, stop=False)
                nc.tensor.matmul(out=lp[:], lhsT=wt_s[:, 1, :], rhs=xsT[:, 1, :],
                                 start=False, stop=True)
                o2 = o2pool.tile([HID, SGRP], bf16, tag="o2")
                nc.scalar.activation(out=o2[:], in_=lp[:],
                                     func=mybir.ActivationFunctionType.Identity,
                                     bias=bb_s[:, 0:1], scale=1.0)
                nc.sync.dma_start(out=o2T_d[:, sg * SGRP : (sg + 1) * SGRP], in_=o2[:])

            def load_oh_chunk(ci):
                if ci * OHC >= PADR or ci in oh_tiles:
                    return
                tile = ohpool.tile([NUM_T, OHC], bf16, tag="oh")
                lo = ci * OHC
                hi = min(lo + OHC, PADR)
                nc.sync.dma_start(out=tile[:, 0 : hi - lo], in_=oh_d[:, lo:hi])
                oh_tiles[ci] = tile

            # prologue: first two oh chunks, first two sel groups' gathers
            load_oh_chunk(0)
            load_oh_chunk(1)
            emit_gathers(0)
            emit_gathers(1)

            sel_triggers = sorted(sel_at)
            sel_i = 0
            for g in range(NG):
                c0 = g * DGRP
                ci = g // (OHC // DGRP)
                if g % (OHC // DGRP) == 0:
                    load_oh_chunk(ci + 2)
                if g % SLABG == 0:
                    oslab = opool.tile([HID, SLABG * DGRP], bf16, tag="oslab")
                    slab_g0 = g

                pd = psd.tile([HID, DGRP], f32, tag="pd")
                nc.tensor.matmul(
                    out=pd[:], lhsT=t2_s[:],
                    rhs=oh_tiles[ci][:, c0 - ci * OHC : c0 - ci * OHC + DGRP],
                    start=True, stop=True,
                )
                so = (g - slab_g0) * DGRP
                if g % 2 == 0:
                    nc.vector.tensor_copy(oslab[:, so : so + DGRP], pd[:])
                else:
                    nc.scalar.activation(out=oslab[:, so : so + DGRP], in_=pd[:],
                                         func=mybir.ActivationFunctionType.Copy)

                if g == slab_g0 + SLABG - 1 or g == NG - 1:
                    lo = slab_g0 * DGRP
                    hi = (g + 1) * DGRP
                    nc.scalar.dma_start(out=outT_d[:, lo:hi], in_=oslab[:, 0 : hi - lo])

                if sel_i < sgroups and g == sel_triggers[sel_i]:
                    if sel_i + 2 < sgroups:
                        emit_gathers(sel_i + 2)
                    emit_sel_compute(sel_i)
                    sel_i += 1

            while sel_i < sgroups:
                if sel_i + 2 < sgroups:
                    emit_gathers(sel_i + 2)
                emit_sel_compute(sel_i)
                sel_i += 1

    nc.compile()
    return nc


def _prepare(inputs):
    x = np.asarray(inputs["x"])
    nt = np.asarray(inputs["node_type"]).astype(np.int64)
    item = int(np.asarray(inputs["item_id"]))
    emb = np.asarray(inputs["emb_weight"], dtype=np.float32)
    W = np.asarray(inputs["W"], dtype=np.float32)
    b = np.asarray(inputs["b"], dtype=np.float32)

    t2 = emb.astype(BF16)
    wt = np.ascontiguousarray(W.T).astype(BF16)
    bb = b.astype(np.float32).reshape(HID, 1)

    sels = []
    max_nsel = 0
    for c in range(NCORES):
        sel = np.flatnonzero(nt[c * NSH : (c + 1) * NSH] == item).astype(np.int32)
        sels.append(sel)
        max_nsel = max(max_nsel, len(sel))
    tsel = max(64, ((-(-max_nsel // 128) + 3) // 4 + 1) * 4)

    in_maps = []
    for c in range(NCORES):
        nt_sh = nt[c * NSH : (c + 1) * NSH]
        xb = np.ascontiguousarray(x[c * NSH : (c + 1) * NSH]).astype(BF16)

        oh = np.zeros((NUM_T, PADR), dtype=BF16)
        for t in range(NUM_T):
            oh[t, :NSH] = (nt_sh == t)

        idxp = np.zeros(tsel * 128, dtype=np.int32)
        idxp[: len(sels[c])] = sels[c]
        idx_pm = np.ascontiguousarray(idxp.reshape(tsel, 128).T)

        in_maps.append({"x": xb, "oh": oh, "idx": idx_pm,
                        "t2": t2, "wt": wt, "bb": bb})
    return tsel, sels, in_maps


def _run(inputs, trace=False):
    _ensure_axon_profile_hook()
    tsel, sels, in_maps = _prepare(inputs)
    if tsel not in _CACHE:
        _CACHE[tsel] = _build(tsel)
    nc = _CACHE[tsel]
    res = run_bass_kernel_spmd(nc, in_maps, core_ids=list(range(NCORES)), trace=trace)
    out = np.empty((N, HID), np.float32)
    for c in range(NCORES):
        outT = res.results[c]["outT"]          # [HID, PADR] bf16
        osh = out[c * NSH : (c + 1) * NSH]
        osh[:] = outT[:, :NSH].astype(np.float32).T
        sel = sels[c]
        if len(sel):
            o2T = res.results[c]["o2T"]        # [HID, tsel*128] bf16
            osh[sel] = o2T[:, : len(sel)].astype(np.float32).T
    return out, res


def kernel(**inputs) -> np.ndarray:
    out, _ = _run(inputs, trace=bool(os.environ.get("KERNEL_TRACE")))
    return out


# revision 9
# speedup vs baseline: 1.2444x; 1.0318x over previous
"""Trainium2 Bass kernel for nn_Node_Transformation.

Reference semantics, for row n:
    out[n] = x[n] @ W.T + b            if node_type[n] == item_id
             emb_weight[node_type[n]]  otherwise

Only ~1/8 of rows take the linear path, so the kernel is split:

  Dense part (all rows): out_dense[n] = emb_weight[node_type[n]], computed as a
  one-hot matmul: outT[h, r] = sum_t table[t, h] * onehot[t, r], with the tiny
  table as the stationary operand and the host-built one-hot indicator
  streaming as rhs. Output is produced hid-major ("outT") so the per-group
  varying operand is the streaming one (no LDWEIGHTS churn).

  Sparse part (selected rows only): row indices where node_type == item_id are
  computed on host (metadata only); the kernel bulk-gathers just those x rows
  with dma_gather (1/8 of x traffic, ~1 Q7 descriptor-gen call per 1024 rows),
  transposes them on the PE, and computes lin = x_sel @ W.T + b into a compact
  second output. The host scatters those rows over the dense result while
  unsharding. dma_gather needs int16 indices, so each shard's x is staged as
  two half tensors (rows < 31250 and >= 31250) with rebased indices; slots are
  padded with index 0 so num_idxs is the same on every core (SPMD).

Everything on-device is bf16 (psum accumulation in f32); the correctness gate
is a scale-relative 2e-2 absmax, bf16 error is ~4e-3.

Sharding: data-parallel over N across 8 NeuronCores; weights/table replicated.
"""

import os
import numpy as np
import ml_dtypes

import concourse.bass as bass
import concourse.bacc as bacc
import concourse.mybir as mybir
from concourse.tile import TileContext
from concourse.bass_utils import run_bass_kernel_spmd
from concourse.masks import make_identity

# ---- problem constants (hardcoded per contest contract) ----
N = 500000
IN_CH = 256
HID = 128
NUM_T = 8
NCORES = 8
NSH = N // NCORES          # 62500 rows per core
HALF = NSH // 2            # 31250: x split so gather indices fit int16
DGRP = 512                 # dense rows per matmul group (one f32 PSUM bank)
NG = (NSH + DGRP - 1) // DGRP          # 123 dense groups
PADR = NG * DGRP                       # 62976 padded rows per core
OHC = 8192                 # one-hot columns loaded per DMA (16 dense groups)
SLABG = 16                 # dense groups per output slab (8192 cols per DMA)
SGRP = 512                 # sel rows per matmul group (4 tiles, f32 psum bank)
GCHUNK = 1024              # sel rows per dma_gather call (2 matmul groups)

BF16 = ml_dtypes.bfloat16

_CACHE = {}


def _ensure_axon_profile_hook():
    """bass_utils' trace path imports antenv.axon_hooks, which this image
    lacks. Register an equivalent module backed by the axon PJRT .so so
    trace=True (or BASS_TRACE=1) works instead of crashing."""
    try:
        import antenv.axon_hooks  # noqa: F401
        return
    except ImportError:
        pass
    import sys
    import types

    hook = None
    try:
        from trn_agent_boot.trn_boot import _ntff_profile_via_ctypes

        hook = _ntff_profile_via_ctypes("/opt/axon/libaxon_pjrt.so")
    except Exception:
        hook = None
    mod = types.ModuleType("antenv.axon_hooks")
    mod.get_axon_ntff_profile_hook = lambda: hook
    mod.set_axon_ntff_profile_hook = lambda h: None
    sys.modules["antenv.axon_hooks"] = mod
    try:
        import antenv

        antenv.axon_hooks = mod
    except ImportError:
        pass


def _build(tlo: int, thi: int) -> bass.Bass:
    """tlo/thi: number of 128-row sel tiles gathered from the low/high half
    of x. Both are multiples of 8 so gathers are whole GCHUNK calls."""
    nc = bacc.Bacc("TRN2")
    f32 = mybir.dt.float32
    bf16 = mybir.dt.bfloat16
    i16 = mybir.dt.int16

    tsel = tlo + thi
    ncalls = tsel * 128 // GCHUNK
    calls_lo = tlo * 128 // GCHUNK
    sgroups = tsel * 128 // SGRP

    xlo_d = nc.dram_tensor("xlo", [HALF, IN_CH], bf16, kind="ExternalInput")
    xhi_d = nc.dram_tensor("xhi", [HALF, IN_CH], bf16, kind="ExternalInput")
    oh_d = nc.dram_tensor("oh", [NUM_T, PADR], bf16, kind="ExternalInput")
    idx_d = nc.dram_tensor("idx", [128, tsel * 8], i16, kind="ExternalInput")
    t2_d = nc.dram_tensor("t2", [NUM_T, HID], bf16, kind="ExternalInput")
    wt_d = nc.dram_tensor("wt", [IN_CH, HID], bf16, kind="ExternalInput")
    bb_d = nc.dram_tensor("bb", [HID, 1], f32, kind="ExternalInput")
    outT_d = nc.dram_tensor("outT", [HID, PADR], bf16, kind="ExternalOutput")
    o2T_d = nc.dram_tensor("o2T", [HID, tsel * 128], bf16, kind="ExternalOutput")

    # spread the sel compute groups evenly through the dense loop
    sel_at = sorted(set(int(round((i + 0.5) * NG / sgroups)) for i in range(sgroups)))
    assert len(sel_at) == sgroups

    with TileContext(nc) as tc:
        with (
            tc.tile_pool(name="singles", bufs=1) as singles,
            tc.tile_pool(name="ohp", bufs=3) as ohpool,
            tc.tile_pool(name="osl", bufs=3) as opool,
            tc.tile_pool(name="xsp", bufs=3) as xpool,
            tc.tile_pool(name="xtp", bufs=2) as xtpool,
            tc.tile_pool(name="o2p", bufs=2) as o2pool,
            tc.tile_pool(name="psd", bufs=3, space="PSUM") as psd,
            tc.tile_pool(name="pst", bufs=3, space="PSUM") as pst,
            tc.tile_pool(name="psl", bufs=2, space="PSUM") as psl,
        ):
            ident = singles.tile([128, 128], bf16)
            make_identity(nc, ident)

            t2_s = singles.tile([NUM_T, HID], bf16)
            nc.sync.dma_start(out=t2_s[:], in_=t2_d[:])
            wt_s = singles.tile([128, 2, HID], bf16)
            nc.sync.dma_start(out=wt_s[:], in_=wt_d[:].rearrange("(k c) h -> c k h", k=2))
            bb_s = singles.tile([HID, 1], f32)
            nc.sync.dma_start(out=bb_s[:], in_=bb_d[:])
            idx_s = singles.tile([128, tsel * 8], i16)
            nc.sync.dma_start(out=idx_s[:], in_=idx_d[:])

            oh_tiles = {}
            oslab = None
            slab_g0 = 0
            gathered = {}          # call index -> xg slab tile

            def emit_gather(k):
                if k >= ncalls or k in gathered:
                    return
                xg = xpool.tile([128, GCHUNK // 128, IN_CH], bf16, tag="xg")
                src = xlo_d if k < calls_lo else xhi_d
                cols = GCHUNK // 16
                nc.gpsimd.dma_gather(
                    out_ap=xg[:],
                    in_ap=src[:],
                    idxs_ap=idx_s[:, k * cols : (k + 1) * cols],
                    num_idxs=GCHUNK,
                    num_idxs_reg=GCHUNK,
                    elem_size=IN_CH,
                )
                gathered[k] = xg

            def emit_sel_compute(sg):
                k = sg * SGRP // GCHUNK
                j0 = (sg * SGRP - k * GCHUNK) // 128
                xg = gathered[k]
                xsT = xtpool.tile([128, 2, SGRP], bf16, tag="xsT")
                for j in range(SGRP // 128):
                    pt = pst.tile([128, 2, 128], bf16, tag="pt")
                    nc.tensor.transpose(pt[:, 0, :], xg[:, j0 + j, 0:128], ident[:])
                    nc.tensor.transpose(pt[:, 1, :], xg[:, j0 + j, 128:256], ident[:])
                    nc.vector.tensor_copy(xsT[:, :, j * 128 : (j + 1) * 128], pt[:])
                if j0 + SGRP // 128 >= GCHUNK // 128:
                    del gathered[k]
                lp = psl.tile([HID, SGRP], f32, tag="lp")
                nc.tensor.matmul(out=lp[:], lhsT=wt_s[:, 0, :], rhs=xsT[:, 0, :],
                                 start=True, stop=False)
                nc.tensor.matmul(out=lp[:], lhsT=wt_s[:, 1, :], rhs=xsT[:, 1, :],
                                 start=False, stop=True)
                o2 = o2pool.tile([HID, SGRP], bf16, tag="o2")
                nc.scalar.activation(out=o2[:], in_=lp[:],
                                     func=mybir.ActivationFunctionType.Identity,
                                     bias=bb_s[:, 0:1], scale=1.0)
                nc.sync.dma_start(out=o2T_d[:, sg * SGRP : (sg + 1) * SGRP], in_=o2[:])

            def load_oh_chunk(ci):
                if ci * OHC >= PADR or ci in oh_tiles:
                    return
                tile = ohpool.tile([NUM_T, OHC], bf16, tag="oh")
                lo = ci * OHC
                hi = min(lo + OHC, PADR)
                nc.sync.dma_start(out=tile[:, 0 : hi - lo], in_=oh_d[:, lo:hi])
                oh_tiles[ci] = tile

            # prologue: first oh chunks and first two gather calls
            load_oh_chunk(0)
            load_oh_chunk(1)
            emit_gather(0)
            emit_gather(1)

            sel_i = 0
            for g in range(NG):
                c0 = g * DGRP
                ci = g // (OHC // DGRP)
                if g % (OHC // DGRP) == 0:
                    load_oh_chunk(ci + 2)
                if g % SLABG == 0:
                    oslab = opool.tile([HID, SLABG * DGRP], bf16, tag="oslab")
                    slab_g0 = g

                pd = psd.tile([HID, DGRP], f32, tag="pd")
                nc.tensor.matmul(
                    out=pd[:], lhsT=t2_s[:],
                    rhs=oh_tiles[ci][:, c0 - ci * OHC : c0 - ci * OHC + DGRP],
                    start=True, stop=True,
                )
                so = (g - slab_g0) * DGRP
                if g % 2 == 0:
                    nc.vector.tensor_copy(oslab[:, so : so + DGRP], pd[:])
                else:
                    nc.scalar.activation(out=oslab[:, so : so + DGRP], in_=pd[:],
                                         func=mybir.ActivationFunctionType.Copy)

                if g == slab_g0 + SLABG - 1 or g == NG - 1:
                    lo = slab_g0 * DGRP
                    hi = (g + 1) * DGRP
                    nc.scalar.dma_start(out=outT_d[:, lo:hi], in_=oslab[:, 0 : hi - lo])

                if sel_i < sgroups and g == sel_at[sel_i]:
                    if sel_i % (GCHUNK // SGRP) == 0:
                        emit_gather(sel_i * SGRP // GCHUNK + 2)
                    emit_sel_compute(sel_i)
                    sel_i += 1

            while sel_i < sgroups:
                if sel_i % (GCHUNK // SGRP) == 0:
                    emit_gather(sel_i * SGRP // GCHUNK + 2)
                emit_sel_compute(sel_i)
                sel_i += 1

    nc.compile()
    return nc


def _pack_idx16(vals: np.ndarray, ntiles: int) -> np.ndarray:
    """Pack a region's rebased indices (padded with 0) into the dma_gather
    index layout: slot i -> [i % 16, i // 16], 16-partition pattern."""
    arr = np.zeros(ntiles * 128, dtype=np.int16)
    arr[: len(vals)] = vals
    return np.ascontiguousarray(arr.reshape(-1, 16).T)     # [16, ntiles*8]


def _prepare(inputs):
    x = np.asarray(inputs["x"])
    nt = np.asarray(inputs["node_type"]).astype(np.int64)
    item = int(np.asarray(inputs["item_id"]))
    emb = np.asarray(inputs["emb_weight"], dtype=np.float32)
    W = np.asarray(inputs["W"], dtype=np.float32)
    b = np.asarray(inputs["b"], dtype=np.float32)

    t2 = emb.astype(BF16)
    wt = np.ascontiguousarray(W.T).astype(BF16)
    bb = b.astype(np.float32).reshape(HID, 1)

    sel_los, sel_his = [], []
    max_lo = max_hi = 0
    for c in range(NCORES):
        sel = np.flatnonzero(nt[c * NSH : (c + 1) * NSH] == item)
        sel_lo = sel[sel < HALF].astype(np.int32)
        sel_hi = (sel[sel >= HALF] - HALF).astype(np.int32)
        sel_los.append(sel_lo)
        sel_his.append(sel_hi)
        max_lo = max(max_lo, len(sel_lo))
        max_hi = max(max_hi, len(sel_hi))
    # tiles per region, multiples of 8 (whole GCHUNK calls), with headroom
    tlo = max(32, -(-max_lo // 128) + 7 & ~7)
    thi = max(32, -(-max_hi // 128) + 7 & ~7)

    in_maps = []
    for c in range(NCORES):
        nt_sh = nt[c * NSH : (c + 1) * NSH]
        xb = x[c * NSH : (c + 1) * NSH].astype(BF16)

        oh = np.zeros((NUM_T, PADR), dtype=BF16)
        for t in range(NUM_T):
            oh[t, :NSH] = (nt_sh == t)

        idx16 = np.concatenate(
            [_pack_idx16(sel_los[c], tlo), _pack_idx16(sel_his[c], thi)], axis=1)
        idx16 = np.ascontiguousarray(np.tile(idx16, (8, 1)))   # [128, tsel*8]

        in_maps.append({"xlo": np.ascontiguousarray(xb[:HALF]),
                        "xhi": np.ascontiguousarray(xb[HALF:]),
                        "oh": oh, "idx": idx16,
                        "t2": t2, "wt": wt, "bb": bb})
    return tlo, thi, sel_los, sel_his, in_maps


def _run(inputs, trace=False):
    _ensure_axon_profile_hook()
    tlo, thi, sel_los, sel_his, in_maps = _prepare(inputs)
    if (tlo, thi) not in _CACHE:
        _CACHE[(tlo, thi)] = _build(tlo, thi)
    nc = _CACHE[(tlo, thi)]
    res = run_bass_kernel_spmd(nc, in_maps, core_ids=list(range(NCORES)), trace=trace)
    out = np.empty((N, HID), np.float32)
    for c in range(NCORES):
        outT = res.results[c]["outT"]          # [HID, PADR] bf16
        osh = out[c * NSH : (c + 1) * NSH]
        osh[:] = outT[:, :NSH].astype(np.float32).T
        o2T = res.results[c]["o2T"]            # [HID, tsel*128] bf16
        lo, hi = sel_los[c], sel_his[c]
        if len(lo):
            osh[lo] = o2T[:, : len(lo)].astype(np.float32).T
        if len(hi):
            base = tlo * 128
            osh[hi + HALF] = o2T[:, base : base + len(hi)].astype(np.float32).T
    return out, res


def kernel(**inputs) -> np.ndarray:
    out, _ = _run(inputs, trace=bool(os.environ.get("KERNEL_TRACE")))
    return out
